# revision 2
# baseline (speedup 1.0000x reference)
"""Trainium2 Bass kernel for CustomPositionsPiecewiseConv2d.

Math: for knots positions=[-1,-.5,0,.5,1] and inputs x in [0,1], the per-value
interpolation coefficients reduce to three active planes (knots 2,3,4):
    c2 = relu(1-2v)
    c4 = max(relu(2v-1), T)        T = 1[v >= theta]  (the isclose(v,1) mask)
    c3 = 1 - c2 - c4
plus the raw-value plane v (contracted with the identity-shortcut mask).
Each plane is an elementwise function of v, and v is just shifted/padded copies
of x, so the planes are computed once per padded image and the 3x3 im2col taps
become 9 shifted access-pattern reads feeding PSUM-accumulated matmuls:
    out[o, l] = sum_{tap k} Wk[(g,c), o].T @ Y[(g,c), shift_k(l)]    (K = 4*32 = 128)

Sharding: data-parallel over batch, 2 images per core on 8 cores.
"""

import numpy as np

B, C, H, W = 16, 32, 64, 64
O, P, KH, KW = 128, 5, 3, 3
NCORES = 8
IPC = B // NCORES            # images per core
HP, WP = H + 2, W + 2        # padded image (pad=1)
RT = 8                       # output rows per L-tile
NT = H // RT                 # L-tiles per image
K2 = KH * KW
ATOL = 1e-5
RTOL = 1e-5

MM_DTYPE = "float32"         # "float32" (exact, 4 cyc/row) or "float32r" (1 cyc/row @ N>=256)


# ---------------------------------------------------------------- host math


def _isclose_np(a, b):
    return np.abs(a - b) <= np.float32(ATOL) + np.float32(RTOL) * np.abs(b)


def _reference_np(x, weights, bias, positions):
    """Direct numpy port of the reference (fallback path)."""
    EPS = 1e-6
    Bn, Cn, Hn, Wn = x.shape
    On, _, Pn, KHn, KWn = weights.shape
    xp = np.pad(x, ((0, 0), (0, 0), (1, 1), (1, 1)))
    cols = [
        xp[:, :, i : i + Hn, j : j + Wn] for i in range(KHn) for j in range(KWn)
    ]
    pat = np.stack(cols, axis=2)                     # [B,C,K2,H,W]
    v = pat.reshape(Bn, Cn, KHn * KWn, Hn * Wn).astype(np.float32)

    left, right = positions[:-1], positions[1:]
    denom = right - left
    denom = np.where(denom == 0, np.float32(EPS), denom)
    varc = (1.0 / denom).astype(np.float32)
    const = (-left * varc).astype(np.float32)

    m_first = _isclose_np(v, positions[0])
    m_last = _isclose_np(v, positions[-1])
    in_range = (~(m_first | m_last)) & (v >= positions[0]) & (v <= positions[-1])

    coeff = np.zeros(v.shape + (Pn,), np.float32)
    coeff[..., 0] += m_first.astype(np.float32)
    coeff[..., Pn - 1] += m_last.astype(np.float32)
    for p in range(Pn - 1):
        m = (in_range & (v >= positions[p]) & (v < positions[p + 1])).astype(
            np.float32
        )
        t = v * varc[p] + const[p]
        coeff[..., p] += m * (1.0 - t)
        coeff[..., p + 1] += m * t

    Wk = np.transpose(weights, (0, 1, 3, 4, 2)).reshape(On, Cn, KHn * KWn, Pn)
    ident = np.all(np.abs(Wk - 1.0) <= np.float32(ATOL + RTOL), axis=-1)
    Wk_eff = np.where(ident[..., None], np.float32(0.0), Wk)

    out = np.einsum("bcklp,ockp->bol", coeff, Wk_eff, optimize=True)
    out = out + np.einsum(
        "bckl,ock->bol", v, ident.astype(np.float32), optimize=True
    )
    out = out + bias[None, :, None]
    return out.reshape(Bn, On, Hn, Wn).astype(np.float32)


def _compute_theta():
    """Smallest fp32 v such that fp32(1-v) <= fp32(ATOL + RTOL*1.0), matching
    the reference's m_last = isclose(v, 1.0) for v <= 1."""
    tau = np.float32(np.float32(ATOL) + np.float32(RTOL) * np.float32(1.0))
    th = np.float32(np.float32(1.0) - tau)
    # walk down while still inside the isclose set
    while np.float32(np.float32(1.0) - np.nextafter(th, np.float32(0.0))) <= tau:
        th = np.nextafter(th, np.float32(0.0))
    # walk up until inside
    while np.float32(np.float32(1.0) - th) > tau:
        th = np.nextafter(th, np.float32(2.0))
    return np.float32(th)


def _build_wstk(weights):
    """lhsT per tap: wstk[g*32+c, k, o]; groups g = (v/ident, c2, c3, c4)."""
    Wk = np.transpose(weights, (0, 1, 3, 4, 2)).reshape(O, C, K2, P)
    ident = np.all(np.abs(Wk - 1.0) <= np.float32(ATOL + RTOL), axis=-1)  # [O,C,K2]
    Wk_eff = np.where(ident[..., None], np.float32(0.0), Wk)
    wstk = np.zeros((4 * C, K2, O), np.float32)
    wstk[0:C] = ident.astype(np.float32).transpose(1, 2, 0)          # v plane
    for g, p in ((1, 2), (2, 3), (3, 4)):
        wstk[g * C : (g + 1) * C] = Wk_eff[:, :, :, p].transpose(1, 2, 0)
    return np.ascontiguousarray(wstk)


# ---------------------------------------------------------------- device IR


def _build_nc(theta):
    import concourse.tile as tile
    from concourse import bacc, mybir

    f32 = mybir.dt.float32
    mm_dt = getattr(mybir.dt, MM_DTYPE)
    Alu = mybir.AluOpType
    Act = mybir.ActivationFunctionType

    nc = bacc.Bacc("TRN2", target_bir_lowering=False, debug=False,
                   num_devices=NCORES)
    x_d = nc.dram_tensor("x", [IPC, C, H, W], f32, kind="ExternalInput").ap()
    w_d = nc.dram_tensor("wstk", [4 * C, K2, O], f32, kind="ExternalInput").ap()
    b_d = nc.dram_tensor("bias", [O, 1], f32, kind="ExternalInput").ap()
    o_d = nc.dram_tensor("out", [IPC, O, H, W], f32, kind="ExternalOutput").ap()

    with tile.TileContext(nc) as tc:
        with (
            tc.tile_pool(name="const", bufs=1) as constp,
            tc.tile_pool(name="plane", bufs=1) as planep,
            tc.tile_pool(name="ybuf", bufs=1) as ybufp,
            tc.tile_pool(name="psum", bufs=4, space="PSUM") as psump,
            tc.tile_pool(name="osb", bufs=4) as osbp,
        ):
            w_sb = constp.tile([4 * C, K2, O], f32)
            nc.sync.dma_start(w_sb[:], w_d[:])
            b_sb = constp.tile([O, 1], f32)
            nc.sync.dma_start(b_sb[:], b_d[:])

            X2 = planep.tile([IPC * C, HP, WP], f32)
            Pc2 = planep.tile([IPC * C, HP, WP], f32)
            Pc3 = planep.tile([IPC * C, HP, WP], f32)
            Pc4 = planep.tile([IPC * C, HP, WP], f32)
            Tm = planep.tile([IPC * C, HP, WP], f32)

            # zero the pad border of the value image
            nc.vector.memset(X2[:, 0, :], 0.0)
            nc.vector.memset(X2[:, HP - 1, :], 0.0)
            nc.vector.memset(X2[:, 1 : HP - 1, 0], 0.0)
            nc.vector.memset(X2[:, 1 : HP - 1, WP - 1], 0.0)
            for i in range(IPC):
                nc.sync.dma_start(
                    X2[i * C : (i + 1) * C, 1 : HP - 1, 1 : WP - 1], x_d[i]
                )

            # coefficient planes (both images at once, 64 partitions)
            negone = constp.tile([IPC * C, 1], f32)
            nc.vector.memset(negone[:], -1.0)
            nc.scalar.activation(Pc2[:], X2[:], Act.Relu, bias=1.0, scale=-2.0)
            nc.scalar.activation(Pc4[:], X2[:], Act.Relu, bias=negone[:], scale=2.0)
            nc.vector.tensor_scalar(Tm[:], X2[:], float(theta), None, Alu.is_ge)
            nc.vector.tensor_tensor(Pc4[:], Pc4[:], Tm[:], Alu.max)
            nc.vector.tensor_tensor(Pc3[:], Pc2[:], Pc4[:], Alu.add)
            nc.vector.tensor_scalar(Pc3[:], Pc3[:], -1.0, 1.0, Alu.mult, Alu.add)

            for i in range(IPC):
                Y = ybufp.tile([4 * C, HP, WP], f32, name=f"Y{i}")
                s = slice(i * C, (i + 1) * C)
                nc.sync.dma_start(Y[0 * C : 1 * C], X2[s])
                nc.sync.dma_start(Y[1 * C : 2 * C], Pc2[s])
                nc.sync.dma_start(Y[2 * C : 3 * C], Pc3[s])
                nc.sync.dma_start(Y[3 * C : 4 * C], Pc4[s])

                for t in range(NT):
                    ps = psump.tile([O, RT * W], f32, name="ps")
                    for kh in range(KH):
                        for kw in range(KW):
                            ki = kh * KW + kw
                            rhs = Y[:, t * RT + kh : t * RT + kh + RT, kw : kw + W]
                            lhsT = w_sb[:, ki, :]
                            if MM_DTYPE != "float32":
                                rhs = rhs.bitcast(mm_dt)
                                lhsT = lhsT.bitcast(mm_dt)
                            nc.tensor.matmul(
                                ps[:], lhsT, rhs,
                                start=(ki == 0), stop=(ki == K2 - 1),
                            )
                    osb = osbp.tile([O, RT * W], f32, name="osb")
                    if t % 2 == 0:
                        nc.scalar.activation(
                            osb[:], ps[:], Act.Identity, bias=b_sb[:, 0:1], scale=1.0
                        )
                    else:
                        nc.vector.tensor_scalar(
                            osb[:], ps[:], b_sb[:, 0:1], None, Alu.add
                        )
                    nc.sync.dma_start(
                        o_d[i, :, t * RT : (t + 1) * RT, :],
                        osb[:].rearrange("o (r w) -> o r w", r=RT),
                    )
    nc.compile()
    return nc


def _prep(inputs):
    x = np.ascontiguousarray(np.asarray(inputs["x"], dtype=np.float32))
    weights = np.ascontiguousarray(np.asarray(inputs["weights"], dtype=np.float32))
    bias = np.ascontiguousarray(np.asarray(inputs["bias"], dtype=np.float32))
    positions = np.ascontiguousarray(
        np.asarray(inputs["positions"], dtype=np.float32)
    )
    return x, weights, bias, positions


def _fast_path_ok(x, positions):
    expect = np.linspace(-1.0, 1.0, P, dtype=np.float32)
    return (
        x.shape == (B, C, H, W)
        and positions.shape == (P,)
        and np.array_equal(positions, expect)
        and float(x.min()) >= 0.0
        and float(x.max()) <= 1.0
    )


def kernel(**inputs):
    x, weights, bias, positions = _prep(inputs)
    if not _fast_path_ok(x, positions):
        return _reference_np(x, weights, bias, positions)

    from concourse.bass_utils import run_bass_kernel_spmd

    theta = _compute_theta()
    wstk = _build_wstk(weights)
    bias2d = np.ascontiguousarray(bias.reshape(O, 1))

    nc = _build_nc(theta)
    in_maps = [
        {"x": np.ascontiguousarray(x[i * IPC : (i + 1) * IPC]),
         "wstk": wstk, "bias": bias2d}
        for i in range(NCORES)
    ]
    res = run_bass_kernel_spmd(nc, in_maps, core_ids=list(range(NCORES)))
    out = np.concatenate([res.results[i]["out"] for i in range(NCORES)], axis=0)
    return np.ascontiguousarray(out)


# ------------------------------------------------------------ dev utilities


def _run_sim(inputs):
    """CoreSim single-core run (images 0..IPC-1) for correctness debugging."""
    from concourse.bass_interp import CoreSim

    x, weights, bias, positions = _prep(inputs)
    assert _fast_path_ok(x, positions)
    nc = _build_nc(_compute_theta())
    sim = CoreSim(nc)
    sim.tensor("x")[:] = x[:IPC]
    sim.tensor("wstk")[:] = _build_wstk(weights)
    sim.tensor("bias")[:] = bias.reshape(O, 1)
    sim.simulate()
    return np.array(sim.tensor("out"))


# revision 6
# speedup vs baseline: 1.8564x; 1.8564x over previous
"""Trainium2 Bass kernel for CustomPositionsPiecewiseConv2d.

Math: for knots positions=[-1,-.5,0,.5,1] and inputs x in [0,1], the per-value
interpolation coefficients reduce to three active planes (knots 2,3,4):
    c2 = relu(1-2v)
    c4 = max(relu(2v-1), T)        T = 1[v >= theta]  (the isclose(v,1) mask)
    c3 = 1 - c2 - c4
plus the raw-value plane v (contracted with the identity-shortcut mask).
Each plane is an elementwise function of v, and v is just shifted/padded copies
of x, so the planes are computed once per padded image and the 3x3 im2col taps
become 9 shifted access-pattern reads feeding PSUM-accumulated matmuls:
    out[o, l] = sum_{tap k} Wk[(g,c), o].T @ Y[(g,c), shift_k(l)]    (K = 4*32 = 128)

Sharding: data-parallel over batch, 2 images per core on 8 cores.
"""

import numpy as np

B, C, H, W = 16, 32, 64, 64
O, P, KH, KW = 128, 5, 3, 3
NCORES = 8
IPC = B // NCORES            # images per core
HP, WP = H + 2, W + 2        # padded image (pad=1)
RT = 8                       # output rows per L-tile
NT = H // RT                 # L-tiles per image
K2 = KH * KW
ATOL = 1e-5
RTOL = 1e-5

MM_DTYPE = "float32"         # "float32" (exact, 4 cyc/row) or "float32r" (1 cyc/row @ N>=256)


# ---------------------------------------------------------------- host math


def _isclose_np(a, b):
    return np.abs(a - b) <= np.float32(ATOL) + np.float32(RTOL) * np.abs(b)


def _reference_np(x, weights, bias, positions):
    """Direct numpy port of the reference (fallback path)."""
    EPS = 1e-6
    Bn, Cn, Hn, Wn = x.shape
    On, _, Pn, KHn, KWn = weights.shape
    xp = np.pad(x, ((0, 0), (0, 0), (1, 1), (1, 1)))
    cols = [
        xp[:, :, i : i + Hn, j : j + Wn] for i in range(KHn) for j in range(KWn)
    ]
    pat = np.stack(cols, axis=2)                     # [B,C,K2,H,W]
    v = pat.reshape(Bn, Cn, KHn * KWn, Hn * Wn).astype(np.float32)

    left, right = positions[:-1], positions[1:]
    denom = right - left
    denom = np.where(denom == 0, np.float32(EPS), denom)
    varc = (1.0 / denom).astype(np.float32)
    const = (-left * varc).astype(np.float32)

    m_first = _isclose_np(v, positions[0])
    m_last = _isclose_np(v, positions[-1])
    in_range = (~(m_first | m_last)) & (v >= positions[0]) & (v <= positions[-1])

    coeff = np.zeros(v.shape + (Pn,), np.float32)
    coeff[..., 0] += m_first.astype(np.float32)
    coeff[..., Pn - 1] += m_last.astype(np.float32)
    for p in range(Pn - 1):
        m = (in_range & (v >= positions[p]) & (v < positions[p + 1])).astype(
            np.float32
        )
        t = v * varc[p] + const[p]
        coeff[..., p] += m * (1.0 - t)
        coeff[..., p + 1] += m * t

    Wk = np.transpose(weights, (0, 1, 3, 4, 2)).reshape(On, Cn, KHn * KWn, Pn)
    ident = np.all(np.abs(Wk - 1.0) <= np.float32(ATOL + RTOL), axis=-1)
    Wk_eff = np.where(ident[..., None], np.float32(0.0), Wk)

    out = np.einsum("bcklp,ockp->bol", coeff, Wk_eff, optimize=True)
    out = out + np.einsum(
        "bckl,ock->bol", v, ident.astype(np.float32), optimize=True
    )
    out = out + bias[None, :, None]
    return out.reshape(Bn, On, Hn, Wn).astype(np.float32)


def _compute_theta():
    """Smallest fp32 v such that fp32(1-v) <= fp32(ATOL + RTOL*1.0), matching
    the reference's m_last = isclose(v, 1.0) for v <= 1."""
    tau = np.float32(np.float32(ATOL) + np.float32(RTOL) * np.float32(1.0))
    th = np.float32(np.float32(1.0) - tau)
    # walk down while still inside the isclose set
    while np.float32(np.float32(1.0) - np.nextafter(th, np.float32(0.0))) <= tau:
        th = np.nextafter(th, np.float32(0.0))
    # walk up until inside
    while np.float32(np.float32(1.0) - th) > tau:
        th = np.nextafter(th, np.float32(2.0))
    return np.float32(th)


def _build_wstk(weights):
    """lhsT per tap: wstk[g*32+c, k, o]; groups g = (v/ident, c2, c3, c4)."""
    Wk = np.transpose(weights, (0, 1, 3, 4, 2)).reshape(O, C, K2, P)
    ident = np.all(np.abs(Wk - 1.0) <= np.float32(ATOL + RTOL), axis=-1)  # [O,C,K2]
    Wk_eff = np.where(ident[..., None], np.float32(0.0), Wk)
    wstk = np.zeros((4 * C, K2, O), np.float32)
    wstk[0:C] = ident.astype(np.float32).transpose(1, 2, 0)          # v plane
    for g, p in ((1, 2), (2, 3), (3, 4)):
        wstk[g * C : (g + 1) * C] = Wk_eff[:, :, :, p].transpose(1, 2, 0)
    return np.ascontiguousarray(wstk)


# ---------------------------------------------------------------- device IR


def _build_nc(theta):
    import concourse.tile as tile
    from concourse import bacc, mybir

    f32 = mybir.dt.float32
    mm_dt = getattr(mybir.dt, MM_DTYPE)
    Alu = mybir.AluOpType
    Act = mybir.ActivationFunctionType
    reduced = MM_DTYPE != "float32"

    nc = bacc.Bacc("TRN2", target_bir_lowering=False, debug=False,
                   num_devices=NCORES)
    x_d = nc.dram_tensor("x", [IPC, C, H, W], f32, kind="ExternalInput").ap()
    w_d = nc.dram_tensor("wstk", [4 * C, K2, O], f32, kind="ExternalInput").ap()
    b_d = nc.dram_tensor("bias", [O, 1], f32, kind="ExternalInput").ap()
    o_d = nc.dram_tensor("out", [IPC, O, H, W], f32, kind="ExternalOutput").ap()

    with tile.TileContext(nc) as tc:
        with (
            tc.tile_pool(name="const", bufs=1) as constp,
            tc.tile_pool(name="plane", bufs=1) as planep,
            tc.tile_pool(name="ybuf", bufs=1) as ybufp,
            tc.tile_pool(name="psum", bufs=4, space="PSUM") as psump,
            tc.tile_pool(name="osb", bufs=4) as osbp,
        ):
            w_sb = constp.tile([4 * C, K2, O], f32)
            nc.sync.dma_start(w_sb[:], w_d[:])
            b_sb = constp.tile([O, 1], f32)
            nc.sync.dma_start(b_sb[:], b_d[:])
            if reduced:
                wr_sb = constp.tile([4 * C, K2, O], mm_dt)
                nc.vector.tensor_copy(wr_sb[:], w_sb[:])
            else:
                wr_sb = w_sb

            X2 = planep.tile([IPC * C, HP, WP], f32)
            Pc2 = planep.tile([IPC * C, HP, WP], mm_dt)
            Pc3 = planep.tile([IPC * C, HP, WP], mm_dt)
            Pc4 = planep.tile([IPC * C, HP, WP], mm_dt)
            Tm = planep.tile([IPC * C, HP, WP], f32)
            if reduced:
                Xr = planep.tile([IPC * C, HP, WP], mm_dt)
            else:
                Xr = X2

            # zero the pad border of the value image
            nc.vector.memset(X2[:, 0, :], 0.0)
            nc.vector.memset(X2[:, HP - 1, :], 0.0)
            nc.vector.memset(X2[:, 1 : HP - 1, 0], 0.0)
            nc.vector.memset(X2[:, 1 : HP - 1, WP - 1], 0.0)
            for i in range(IPC):
                nc.sync.dma_start(
                    X2[i * C : (i + 1) * C, 1 : HP - 1, 1 : WP - 1], x_d[i]
                )

            # coefficient planes (both images at once, 64 partitions)
            negone = constp.tile([IPC * C, 1], f32)
            nc.vector.memset(negone[:], -1.0)
            nc.scalar.activation(Pc2[:], X2[:], Act.Relu, bias=1.0, scale=-2.0)
            nc.scalar.activation(Pc4[:], X2[:], Act.Relu, bias=negone[:], scale=2.0)
            nc.vector.tensor_scalar(Tm[:], X2[:], float(theta), None, Alu.is_ge)
            nc.vector.tensor_tensor(Pc4[:], Pc4[:], Tm[:], Alu.max)
            nc.vector.tensor_tensor(Pc3[:], Pc2[:], Pc4[:], Alu.add)
            nc.vector.tensor_scalar(Pc3[:], Pc3[:], -1.0, 1.0, Alu.mult, Alu.add)
            if reduced:
                nc.vector.tensor_copy(Xr[:], X2[:])

            for i in range(IPC):
                Y = ybufp.tile([4 * C, HP, WP], mm_dt, name=f"Y{i}")
                s = slice(i * C, (i + 1) * C)
                nc.sync.dma_start(Y[0 * C : 1 * C], Xr[s])
                nc.sync.dma_start(Y[1 * C : 2 * C], Pc2[s])
                nc.sync.dma_start(Y[2 * C : 3 * C], Pc3[s])
                nc.sync.dma_start(Y[3 * C : 4 * C], Pc4[s])

                for t in range(NT):
                    ps = psump.tile([O, RT * W], f32, name="ps")
                    for kh in range(KH):
                        for kw in range(KW):
                            ki = kh * KW + kw
                            rhs = Y[:, t * RT + kh : t * RT + kh + RT, kw : kw + W]
                            lhsT = wr_sb[:, ki, :]
                            nc.tensor.matmul(
                                ps[:], lhsT, rhs,
                                start=(ki == 0), stop=(ki == K2 - 1),
                            )
                    osb = osbp.tile([O, RT * W], f32, name="osb")
                    if t % 2 == 0:
                        nc.scalar.activation(
                            osb[:], ps[:], Act.Identity, bias=b_sb[:, 0:1], scale=1.0
                        )
                    else:
                        nc.vector.tensor_scalar(
                            osb[:], ps[:], b_sb[:, 0:1], None, Alu.add
                        )
                    nc.sync.dma_start(
                        o_d[i, :, t * RT : (t + 1) * RT, :],
                        osb[:].rearrange("o (r w) -> o r w", r=RT),
                    )
    nc.compile()
    return nc


def _prep(inputs):
    x = np.ascontiguousarray(np.asarray(inputs["x"], dtype=np.float32))
    weights = np.ascontiguousarray(np.asarray(inputs["weights"], dtype=np.float32))
    bias = np.ascontiguousarray(np.asarray(inputs["bias"], dtype=np.float32))
    positions = np.ascontiguousarray(
        np.asarray(inputs["positions"], dtype=np.float32)
    )
    return x, weights, bias, positions


def _fast_path_ok(x, positions):
    expect = np.linspace(-1.0, 1.0, P, dtype=np.float32)
    return (
        x.shape == (B, C, H, W)
        and positions.shape == (P,)
        and np.array_equal(positions, expect)
        and float(x.min()) >= 0.0
        and float(x.max()) <= 1.0
    )


def kernel(**inputs):
    x, weights, bias, positions = _prep(inputs)
    if not _fast_path_ok(x, positions):
        return _reference_np(x, weights, bias, positions)

    from concourse.bass_utils import run_bass_kernel_spmd

    theta = _compute_theta()
    wstk = _build_wstk(weights)
    bias2d = np.ascontiguousarray(bias.reshape(O, 1))

    nc = _build_nc(theta)
    in_maps = [
        {"x": np.ascontiguousarray(x[i * IPC : (i + 1) * IPC]),
         "wstk": wstk, "bias": bias2d}
        for i in range(NCORES)
    ]
    res = run_bass_kernel_spmd(nc, in_maps, core_ids=list(range(NCORES)))
    out = np.concatenate([res.results[i]["out"] for i in range(NCORES)], axis=0)
    return np.ascontiguousarray(out)


# ------------------------------------------------------------ dev utilities


def _run_sim(inputs):
    """CoreSim single-core run (images 0..IPC-1) for correctness debugging."""
    from concourse.bass_interp import CoreSim

    x, weights, bias, positions = _prep(inputs)
    assert _fast_path_ok(x, positions)
    nc = _build_nc(_compute_theta())
    sim = CoreSim(nc)
    sim.tensor("x")[:] = x[:IPC]
    sim.tensor("wstk")[:] = _build_wstk(weights)
    sim.tensor("bias")[:] = bias.reshape(O, 1)
    sim.simulate()
    return np.array(sim.tensor("out"))
